# revision 46
# baseline (speedup 1.0000x reference)
"""CondConv2d Trainium2 kernel (fp8 DoubleRow implicit-GEMM).

Per-sample expert-combined 3x3 conv (B=16, 256->256 ch, 64x64, fp32),
data-parallel over batch on 8 NeuronCores (2 samples/core).

Math: out = W_b * x with W_b = sum_e r_be bank_e. Computed as three fp8
e4m3 DoubleRow passes (PE cost 0.5 cycles/row, 256-deep contraction):

    out ~= Wh*xh + Wh*xl + Wl*xh

where xh = fp8(16 x), xl = fp8(16 x - xh) (host-prepped) and
Wh = fp8(W128), Wl = fp8(W128 - Wh) split on device from the bf16
combine (W128 = combine of the bank pre-scaled by 128; the scales keep
both operands out of e4m3's subnormal range). The dropped Wl*xl term
and the fp8 rounding of the residuals leave ~4e-3 relative error
(tolerance 2e-2). Host multiplies the bf16 output by 2^-11 (exact).

Conv uses a flattened shared-pad layout: x is stored per partition as a
flat 65-wide image (64 data + 1 zero pad per row, plus top/bottom pad
rows) so each 3x3 tap is a contiguous 1D window at offset 65*kh+kw and
every matmul is a uniform full-width [p, 2, N] DoubleRow operand. One
row's pad column doubles as right pad of row r and left pad of row
r+1, so no device-side padding/copy work exists at all.

Schedule: the bank streams kk-sliced ([cot, kk, ct] DMA tiles) so the
(b0, cot0) expert combine runs on the PE one kernel-position at a time
right behind the DMA, and the first conv PSUM block starts ~6us in,
interleaved kk-major with that combine. The other three (b, cot)
quarters are combined on the DVE via bf16 FMA chains (fp32 scalar r)
while the PE convolves; ACT evicts fp8 hi parts, DVE forms fp8 lo
residuals. b1/cot0's chains are split into two kk-halves so they start
before the cot0 stream finishes and meet the conv's deadline.
"""

import os

import numpy as np
import ml_dtypes

import concourse.tile as tile
from concourse import bacc, mybir
from concourse.bass_utils import run_bass_kernel_spmd

B, C_IN, C_OUT, H, W = 16, 256, 256, 64, 64
KH = KW = 3
KK = KH * KW
E = 8
N_CORES = 8
BPC = B // N_CORES  # samples per core

CI_T = C_IN // 128
CO_T = C_OUT // 128

WROW = W + 1          # flat row pitch (64 data + 1 shared pad)
XFLAT = 1 + WROW + H * WROW + WROW + 1  # 4292
XBASE = 1 + WROW      # flat index of image pixel (0, 0)
GROWS = 7             # output rows per conv PSUM group (7*65=455 <= 512)
NGRP = 10             # 9 full groups + 1 single-row group

XS = 16.0             # x quantization scale (power of 2)
WS = 128.0            # bank/W quantization scale (power of 2)
OUT_DESCALE = 1.0 / (XS * WS)  # applied host-side (exact in fp32)

F32 = mybir.dt.float32
BF16 = mybir.dt.bfloat16
F8 = mybir.dt.float8e4
Alu = mybir.AluOpType
DR = mybir.MatmulPerfMode.DoubleRow

NP_F8 = ml_dtypes.float8_e4m3
NP_BF16 = ml_dtypes.bfloat16

LAST_RESULTS = None  # stashed BassKernelResults for test harness introspection
_NC_CACHE = []


def _rhs_base(g):
    """rhs flat start offset for conv group g at kernel tap (0, 0)."""
    return XBASE + g * GROWS * WROW - WROW - 1


def _grp_width(g):
    return (min(GROWS, H - g * GROWS)) * WROW


def _build():
    nc = bacc.Bacc("TRN2", target_bir_lowering=False, debug=False, enable_asserts=False)
    x_d = nc.dram_tensor("xq", [BPC, 128, 4, XFLAT], F8, kind="ExternalInput")
    # bank: [cot, kk, ct, ci128, e, co128] so one DMA tile = one (cot, kk, ct)
    bank_d = nc.dram_tensor("bank", [CO_T, KK, CI_T, 128, E, 128], BF16, kind="ExternalInput")
    rout_d = nc.dram_tensor("rout", [128, BPC * E], F32, kind="ExternalInput")
    sid_d = nc.dram_tensor("sid", [128, E * 128], BF16, kind="ExternalInput")
    out_d = nc.dram_tensor("out", [BPC, CO_T, 128, H, W], BF16, kind="ExternalOutput")

    with tile.TileContext(nc) as tc:
        with (
            tc.tile_pool(name="const", bufs=1) as constp,
            tc.tile_pool(name="xq", bufs=1) as xqp,
            tc.tile_pool(name="wq", bufs=1) as wqp,
            tc.tile_pool(name="bankt", bufs=1) as bankp,
            tc.tile_pool(name="outs", bufs=10) as outsp,
            tc.tile_pool(name="ctmp", bufs=3) as ctmpp,
            tc.tile_pool(name="psc", bufs=4, space="PSUM") as pscp,
            tc.tile_pool(name="psv", bufs=4, space="PSUM") as psvp,
        ):
            rout = constp.tile([128, BPC * E], F32, tag="rout")

            # host-built bf16 diag(r_b0,e) bank for the PE-side combine
            sid = constp.tile([128, E * 128], BF16, tag="sid")
            nc.sync.dma_start(sid[:], sid_d[:])

            # fp8 combined weights, hi + lo residual, DoubleRow layout
            # [ci128, ci-block 2, kk 9, co 128] per (sample, co-half).
            whi = {}
            wlo = {}
            for b in range(BPC):
                for cot in range(CO_T):
                    whi[(b, cot)] = wqp.tile(
                        [128, CI_T, KK, 128], F8, tag=f"whi{b}{cot}", name=f"whi{b}{cot}"
                    )
                    wlo[(b, cot)] = wqp.tile(
                        [128, CI_T, KK, 128], F8, tag=f"wlo{b}{cot}", name=f"wlo{b}{cot}"
                    )
            # bf16 DVE accumulators for the DVE-combined quarters
            wtmp = {
                (b, cot, ct): wqp.tile(
                    [128, KK * 128], BF16, tag=f"wt{b}{cot}{ct}", name=f"wt{b}{cot}{ct}"
                )
                for b in range(BPC)
                for cot in range(CO_T)
                for ct in range(CI_T)
                if not (b == 0 and cot == 0)
            }

            # Resident bank tiles [ci128, kk, e, co128] per (cot, ct).
            bkt = {
                (cot, ct): bankp.tile(
                    [128, KK, E, 128], BF16, tag=f"bk{cot}{ct}", name=f"bk{cot}{ct}"
                )
                for cot in range(CO_T)
                for ct in range(CI_T)
            }

            xq = {
                b: xqp.tile([128, 4, XFLAT], F8, tag=f"xq{b}", name=f"xq{b}")
                for b in range(BPC)
            }

            def dma_bank(cot, kk, ct):
                nc.sync.dma_start(bkt[(cot, ct)][:, kk, :, :], bank_d[cot, kk, ct])

            def dma_x(b, lo, hi):
                nc.sync.dma_start(xq[b][:, :, lo:hi], x_d[b, :, :, lo:hi])

            # ---- DMA issue order (SP runs these in order; transfers are the
            # pacing resource). x band for conv groups 0-1 first, then the
            # kk-sliced cot0 bank, then the rest of x, then cot1.
            XB0 = 912  # kk0 windows of conv groups 0-1
            dma_bank(0, 0, 0)
            dma_bank(0, 0, 1)
            dma_x(0, 0, XB0)
            nc.sync.dma_start(rout[:], rout_d[:])
            for kk in range(1, KK):
                for ct in range(CI_T):
                    dma_bank(0, kk, ct)
                if kk == 3:
                    dma_x(0, XB0, 2 * XB0)
            dma_x(0, 2 * XB0, 2400)
            dma_x(0, 2400, 2992)
            dma_x(0, 2992, 3650)
            dma_x(0, 3650, XFLAT)
            dma_x(1, 0, 1073)
            dma_x(1, 1073, 2146)
            dma_x(1, 2146, 3219)
            dma_x(1, 3219, XFLAT)
            for kk in range(KK):
                for ct in range(CI_T):
                    dma_bank(1, kk, ct)

            # ---- (b0, cot0) combine on the PE, one kk-slice at a time ----
            def pe_combine_kk(kk):
                for ct in range(CI_T):
                    pc = pscp.tile([128, 128], F32, tag="psc", name="psc")
                    for e in range(E):
                        nc.tensor.matmul(
                            pc[:],
                            sid[:, e * 128 : (e + 1) * 128],
                            bkt[(0, ct)][:, kk, e, :],
                            start=(e == 0),
                            stop=(e == E - 1),
                        )
                    hi = whi[(0, 0)][:, ct, kk, :]
                    lo = wlo[(0, 0)][:, ct, kk, :]
                    nc.scalar.copy(hi, pc[:])
                    nc.vector.scalar_tensor_tensor(
                        lo, pc[:], 1.0, hi, Alu.mult, Alu.subtract
                    )

            # ---- DVE combine for a (b, cot) quarter over a kk range.
            # tensor_scalar (4x DVE mode) + tensor_tensor add (2x mode) beat
            # a chain of scalar_tensor_tensor FMAs (1x) by ~1.5x.
            def dve_combine(b, cot, k0, k1):
                for ct in range(CI_T):
                    wt = wtmp[(b, cot, ct)]
                    dst = wt[:, k0 * 128 : k1 * 128]
                    w = (k1 - k0) * 128
                    for e in range(E):
                        rsc = rout[:, b * E + e : b * E + e + 1]
                        src = bkt[(cot, ct)][:, k0:k1, e, :]
                        if e == 0:
                            nc.vector.tensor_scalar_mul(dst, src, rsc)
                        else:
                            tmp = ctmpp.tile([128, KK * 128], BF16, tag="ctmp", name="ctmp")
                            nc.vector.tensor_scalar_mul(tmp[:, 0:w], src, rsc)
                            nc.vector.tensor_add(dst, dst, tmp[:, 0:w])

            def dve_split(b, cot):
                # both casts on the DVE: ACT is head-of-line blocked by conv
                # evictions, and GPSIMD ucode cannot produce fp8 outputs
                for ct in range(CI_T):
                    src = wtmp[(b, cot, ct)][:].rearrange("p (k co) -> p k co", k=KK)
                    hi = whi[(b, cot)][:, ct, :, :]
                    lo = wlo[(b, cot)][:, ct, :, :]
                    nc.vector.tensor_scalar_mul(hi, src, 1.0)
                    nc.vector.scalar_tensor_tensor(
                        lo, src, 1.0, hi, Alu.mult, Alu.subtract
                    )

            # ---- conv for one (b, cot) quarter over a block of groups ----
            def conv_block(b, cot, groups, pcs):
                terms = (
                    (whi[(b, cot)], 0),
                    (whi[(b, cot)], 2),
                    (wlo[(b, cot)], 0),
                )
                for kk in range(KK):
                    kh, kw = divmod(kk, KW)
                    for ti, (wt, bp) in enumerate(terms):
                        step = kk * 3 + ti
                        for g in groups:
                            width = _grp_width(g)
                            off = _rhs_base(g) + kh * WROW + kw
                            nc.tensor.matmul(
                                pcs[g][:, 0:width],
                                wt[:, :, kk, :],
                                xq[b][:, bp : bp + 2, off : off + width],
                                start=(step == 0),
                                stop=(step == 3 * KK - 1),
                                perf_mode=DR,
                            )

            def evict_gblock(b, cot, groups, pcs):
                # one staging tile + one out-DMA for the whole block
                rows = [min(GROWS, H - g * GROWS) for g in groups]
                tot = sum(rows)
                ot = outsp.tile([128, 3 * GROWS * W], BF16, tag="outs", name="outs")
                r0 = 0
                for g, nrows in zip(groups, rows):
                    nc.scalar.copy(
                        ot[:, r0 * W : (r0 + nrows) * W].rearrange(
                            "p (h w) -> p h w", h=nrows
                        ),
                        pcs[g][:, 0 : nrows * WROW].rearrange(
                            "p (h w) -> p h w", h=nrows
                        )[:, :, 0:W],
                    )
                    r0 += nrows
                h0 = groups[0] * GROWS
                nc.sync.dma_start(
                    out_d[b, cot, :, h0 : h0 + tot, :],
                    ot[:, 0 : tot * W].rearrange("p (h w) -> p h w", h=tot),
                )

            GBLOCKS = [(0, 1), (2,), (3,), (4,), (5,), (6,), (7,), (8,), (9,)]

            def conv_quarter(b, cot, interleave_combine, last=False):
                if last:
                    # keep the final evict+DMA chain short: last PSUM block is
                    # the single-row group, and blocks (8,) and (9,) share one
                    # staging tile + out-DMA
                    tail_pcs = {}
                    for gi, groups in enumerate([(0, 1), (2, 3), (4, 5), (6, 7)]):
                        pcs = {
                            g: psvp.tile([128, GROWS * WROW], F32, tag="psv", name="psv")
                            for g in groups
                        }
                        conv_block(b, cot, groups, pcs)
                        evict_gblock(b, cot, groups, pcs)
                    for g in (8, 9):
                        tail_pcs[g] = psvp.tile(
                            [128, GROWS * WROW], F32, tag="psv", name="psv"
                        )
                        conv_block(b, cot, (g,), tail_pcs)
                    ot = outsp.tile([128, 3 * GROWS * W], BF16, tag="outs", name="outs")
                    # g8 on ACT (runs during g9's matmuls), g9's single row on
                    # the (idle-by-now) DVE so the final chain is short
                    nc.scalar.copy(
                        ot[:, 0 : GROWS * W].rearrange("p (h w) -> p h w", h=GROWS),
                        tail_pcs[8][:, 0 : GROWS * WROW].rearrange(
                            "p (h w) -> p h w", h=GROWS
                        )[:, :, 0:W],
                    )
                    nc.vector.tensor_scalar_mul(
                        ot[:, GROWS * W : (GROWS + 1) * W],
                        tail_pcs[9][:, 0:W],
                        1.0,
                    )
                    nc.sync.dma_start(
                        out_d[b, cot, :, 8 * GROWS : H, :],
                        ot[:, 0 : (GROWS + 1) * W].rearrange(
                            "p (h w) -> p h w", h=GROWS + 1
                        ),
                    )
                    return
                for gi, groups in enumerate(GBLOCKS):
                    pcs = {
                        g: psvp.tile([128, GROWS * WROW], F32, tag="psv", name="psv")
                        for g in groups
                    }
                    if interleave_combine and gi == 0:
                        # Stream the (b0, cot0) combine kk-by-kk right behind
                        # the bank DMA, feeding this first conv block. The
                        # combine runs one kk ahead of the conv so the
                        # hi/lo-split latency is hidden, and the b1/cot0 DVE
                        # chain halves are slotted into the DVE stream's
                        # DMA-paced idle gaps.
                        terms = (
                            (whi[(b, cot)], 0),
                            (whi[(b, cot)], 2),
                            (wlo[(b, cot)], 0),
                        )
                        pe_combine_kk(0)
                        pe_combine_kk(1)
                        for kk in range(KK):
                            if kk + 2 < KK:
                                pe_combine_kk(kk + 2)
                            kh, kw = divmod(kk, KW)
                            for ti, (wt, bp) in enumerate(terms):
                                step = kk * 3 + ti
                                for g in groups:
                                    width = _grp_width(g)
                                    off = _rhs_base(g) + kh * WROW + kw
                                    nc.tensor.matmul(
                                        pcs[g][:, 0:width],
                                        wt[:, :, kk, :],
                                        xq[b][:, bp : bp + 2, off : off + width],
                                        start=(step == 0),
                                        stop=(step == 3 * KK - 1),
                                        perf_mode=DR,
                                    )
                        dve_combine(1, 0, 0, 5)
                        dve_combine(1, 0, 5, 9)
                        dve_split(1, 0)
                    else:
                        conv_block(b, cot, groups, pcs)
                    evict_gblock(b, cot, groups, pcs)

            # Quarter (b0, cot0) first: its per-kk PE combine + ACT/DVE
            # splits lead every in-order engine stream, with b1/cot0's DVE
            # chains slotted in. The cot1 chains follow on the DVE while the
            # PE convolves cot0.
            conv_quarter(0, 0, True)
            dve_combine(0, 1, 0, 9)
            dve_split(0, 1)
            dve_combine(1, 1, 0, 9)
            dve_split(1, 1)
            conv_quarter(1, 0, False)
            conv_quarter(0, 1, False)
            conv_quarter(1, 1, False, last=True)
    nc.compile()
    return nc


def _prep_inputs(x, routing_weights, expert_weight):
    x = np.asarray(x, dtype=np.float32)
    r = np.asarray(routing_weights, dtype=np.float32)
    bank = np.asarray(expert_weight, dtype=np.float32)

    # x -> scaled fp8 hi/lo pair in the flat shared-pad layout
    xs = x * XS
    xh = xs.astype(NP_F8)
    xl = (xs - xh.astype(np.float32)).astype(NP_F8)
    xflat = np.zeros((B, 128, 4, XFLAT), dtype=NP_F8)
    body = xflat[:, :, :, XBASE : XBASE + H * WROW].reshape(B, 128, 4, H, WROW)
    for blk in range(4):
        src = xh if blk < 2 else xl
        ci0 = (blk % 2) * 128
        body[:, :, blk, :, 0:W] = src[:, ci0 : ci0 + 128, :, :]

    # bank -> [cot, kk, ct, ci128, e, co128] bf16, scaled by WS
    # original: [e, co(256), ci(256), kk]
    bk = (bank.reshape(E, CO_T, 128, CI_T, 128, KK) * WS).astype(np.float32)
    bank_t = np.ascontiguousarray(bk.transpose(1, 5, 3, 4, 0, 2)).astype(NP_BF16)
    return xflat, r, bank_t


def kernel(x, routing_weights, expert_weight):
    global LAST_RESULTS
    xflat, r, bank_t = _prep_inputs(x, routing_weights, expert_weight)

    if not _NC_CACHE:
        _NC_CACHE.append(_build())
    nc = _NC_CACHE[0]

    in_maps = []
    for c in range(N_CORES):
        rows = r[c * BPC : (c + 1) * BPC].reshape(BPC * E)
        rbc = np.ascontiguousarray(np.broadcast_to(rows[None, :], (128, BPC * E)))
        sid = np.zeros((128, E, 128), dtype=NP_BF16)
        idx = np.arange(128)
        for e in range(E):
            sid[idx, e, idx] = r[c * BPC, e].astype(NP_BF16)
        in_maps.append(
            {
                "xq": np.ascontiguousarray(xflat[c * BPC : (c + 1) * BPC]),
                "bank": bank_t,
                "rout": rbc,
                "sid": sid.reshape(128, E * 128),
            }
        )

    trace = bool(os.environ.get("KERNEL_TRACE"))
    try:
        res = run_bass_kernel_spmd(
            nc, in_maps, core_ids=list(range(N_CORES)), trace=trace
        )
    except ModuleNotFoundError:
        if not trace:
            raise
        res = run_bass_kernel_spmd(
            nc, in_maps, core_ids=list(range(N_CORES)), trace=False
        )
    LAST_RESULTS = res
    outs = []
    for rr in res.results:
        o = np.asarray(rr["out"]).astype(np.float32) * OUT_DESCALE
        outs.append(o.reshape(BPC, C_OUT, H, W))
    return np.concatenate(outs, axis=0)
